# revision 31
# baseline (speedup 1.0000x reference)
"""Trainium2 Bass kernel for nn_InterViews (retrieval_knn).

Computes, per batch item b: the variance (ddof=1) of the strict-upper-
triangular entries of the cosine-similarity Gram matrix between the
item's V=16 views, negated.

Strategy (data-parallel over bs across 8 cores, 128 items/core):
  - Host: shard rows so core k holds 16 groups of 8 items (each group =
    128 rows = 8 items x 16 views), cast to fp8 e4m3, and pre-transpose
    to xh[c, g, j, v] = x[g*128+v, j*128+c] so the device DMA is pure
    streaming.
  - Device DMA: one merged-const DMA, then the 16 groups as singles
    (group 0 split in 4 slices, group 1 in 2, to cut the first-gram
    latency), all on the sync HWDGE queue; every group tile stays
    SBUF-resident so nothing is gated on buffer reuse and the 16 DMA
    engines stream the 8 MB input back-to-back (~22.5 us).  Dummy fp8
    matmuls bridge the PE's DVFS ramp (0.65 -> 1.2 -> 2.4 GHz over 3 us
    of continuous execution) while the first group is in flight.
  - Grams: per group 16 fp8 DoubleRow matmuls (256 channels each)
    accumulate G = A A^T in fp32 PSUM; 4 quads ride a 4-deep PSUM ring.
  - Postproc per CHUNK (4,4,4,2,1,1 groups — quad chunks mid-stream
    where big DVE ops amortize their fixed cost, tiny chunks at the end
    so the post-stream tail chain is one group deep):
      n2   = rowsum(G * I)  (DVE mul + grouped reduce), rec = 1/n2
      inv  = sqrt(rec) (Act), xd = I*inv (DVE, fp32r)
      ips  = BDO^T @ xd     (PE, fp32r; invT pattern), Act copy to SBUF
      tmp  = G * invT (DVE), t1 = grouped rowsum (DVE)
      wst  = tmp^2 with fused rowsum -> r2 (Act Square + accum)
      stats = (t1*inv, r2*rec) (DVE, fp32r)
      [s1',s2'] = BD^T @ stats  (PE, fp32r)
      out = ((s1')*S1SCL)^2 - s2'/238 via two activations -> y DMA.
"""

import numpy as np

try:
    import concourse.bass as bass  # noqa: F401
except ImportError:  # container installs the repo at /opt/trn_rl_repo
    import sys

    sys.path.insert(0, "/opt/trn_rl_repo")

import ml_dtypes

import concourse.bass as bass
import concourse.mybir as mybir
import concourse.tile as tile
from concourse import bacc
from concourse.bass_utils import run_bass_kernel_spmd

F32 = mybir.dt.float32
F32R = mybir.dt.float32r
BF16 = mybir.dt.bfloat16
F8 = mybir.dt.float8e4
NP_F8 = ml_dtypes.float8_e4m3
P = 128          # partitions / rows per group
C = 4096         # channels
V = 16           # views per item
NCORES = 8
BS = 1024        # total batch
BS_CORE = BS // NCORES   # 128 items per core
IPG = P // V             # 8 items per group
NG = BS_CORE // IPG      # 16 groups per core
NCH = C // P             # 32 channel chunks
QG = 4                   # groups per gram PSUM quad
NQ = NG // QG

# Postproc chunks: steady-state quads, then 2+1+1 so the tail chain
# after the last gram is only one group deep.
CHUNKS = [(0, 1, 2, 3), (4, 5, 6, 7), (8, 9, 10, 11), (12, 13), (14,), (15,)]

MULT = mybir.AluOpType.mult
ADD = mybir.AluOpType.add
AF = mybir.ActivationFunctionType
AXX = mybir.AxisListType.X
DR = mybir.MatmulPerfMode.DoubleRow

S1SCL = 1.0 / np.sqrt(240.0 * 238.0)

NWARM_BIG = 3     # [128, 512] fp8 warmup matmuls (~0.5 us each)
NWARM_SMALL = 5   # [128, 128] fp8 warmup matmuls


def build_tile_kernel(tc, outs, ins):
    """ins = [xh [P, NG, NCH, P] f8e4, cst [P, 3P] f32 = [I | BD-I | BD]]
    outs = [y [IPG, NG] f32]  (y[b, g] = result for local item g*8+b)
    """
    nc = tc.nc
    xh, cst = ins
    (y,) = outs

    from contextlib import ExitStack

    with ExitStack() as ctx:
        x_pool = ctx.enter_context(tc.tile_pool(name="x", bufs=NG))
        g_psum = ctx.enter_context(tc.tile_pool(name="gp", bufs=NQ, space="PSUM"))
        pp_psum = ctx.enter_context(tc.tile_pool(name="pp", bufs=2, space="PSUM"))
        j_psum = ctx.enter_context(tc.tile_pool(name="jp", bufs=1, space="PSUM"))
        ck_pool = ctx.enter_context(tc.tile_pool(name="ckp", bufs=3))
        gh_pool = ctx.enter_context(tc.tile_pool(name="ghp", bufs=3))
        sm_pool = ctx.enter_context(tc.tile_pool(name="sm", bufs=2))
        c_pool = ctx.enter_context(tc.tile_pool(name="const", bufs=1))

        jscr = j_psum.tile([32, 32], F32)

        # Warmup scratch; output borrows the first slot of the gram PSUM
        # ring (quad 3 reuses it via same-engine ordering, no extra wait).
        wsrc = c_pool.tile([P, 4, P], F8)
        nc.vector.memset(wsrc[:], 0.0)
        wdst = g_psum.tile([P, QG * P], F32, name="gps", tag="gps")

        # Consts first (one ~0.2 us DMA), then the groups, all on sync.
        cstt = c_pool.tile([P, 3 * P], F32)
        nc.sync.dma_start(cstt[:], cst[:, :])
        ident = cstt[:, 0:P]
        # BD masks are 0/1 — exact in bf16; bf16 matmuls run at the PE
        # 1-cycle/row fast path (fp32 needs 2 half-speed passes).
        bdrr = c_pool.tile([P, 2 * P], BF16)
        nc.scalar.copy(bdrr[:], cstt[:, P:3 * P])
        bdot = bdrr[:, 0:P]
        bdt = bdrr[:, P:2 * P]

        # 16 single-group DMAs in consumption order (measured: steady
        # ~1.35 us/group completion pace; slicing or pairing disturbs
        # the stream).
        xtiles = [None] * NG
        for g in range(NG):
            xg = x_pool.tile([P, NCH, P], F8, tag="x")
            nc.sync.dma_start(xg[:, :, :], xh[:, g, :, :])
            xtiles[g] = xg

        # PE warmup: runs while group 0 is still in flight.
        for i in range(NWARM_BIG):
            nc.tensor.matmul(wdst[:], wsrc[:, 0, :], wsrc[:],
                             skip_group_check=True)
        for i in range(NWARM_SMALL):
            nc.tensor.matmul(wdst[:, 0:P], wsrc[:, 0, :], wsrc[:, 0, :],
                             skip_group_check=True)
        # Absorb the const-DMA wait into PE's observed clock (TRN2
        # Matmult carries at most one semaphore wait).
        nc.tensor.matmul(jscr[:], cstt[0:32, 0:32], cstt[0:32, 0:32],
                         skip_group_check=True)

        stage = c_pool.tile([P, NG], F32)

        gtiles = [None] * NQ
        cstate = {}           # chunk -> dict(xd=, stats=)

        def gram_group(g):
            q, gl = divmod(g, QG)
            if gl == 0:
                gtiles[q] = g_psum.tile([P, QG * P], F32, name="gps", tag="gps")
            gps = gtiles[q]
            xg = xtiles[g]
            for j in range(NCH // 2):
                nc.tensor.matmul(
                    gps[:, gl * P:(gl + 1) * P],
                    xg[:, 2 * j:2 * j + 2, :],
                    xg[:, 2 * j:2 * j + 2, :],
                    start=(j == 0),
                    stop=(j == NCH // 2 - 1),
                    perf_mode=DR,
                    skip_group_check=True,
                )

        def grams(q):
            for gl in range(QG):
                gram_group(q * QG + gl)

        def post_a(ci):
            """Chunk postproc part A, issued once the chunk's grams are
            done: n2 = diag(G) via mask-mul + grouped rowsum, rec = 1/n2
            (DVE), inv = sqrt(rec) (Act), xd = I*inv (DVE, fp32r)."""
            ch = CHUNKS[ci]
            nl = len(ch)
            q = ch[0] // QG
            gsl = gtiles[q][:, (ch[0] % QG) * P:((ch[0] % QG) + nl) * P]
            st = cstate.setdefault(ci, {})
            idb = ident.unsqueeze(1).broadcast_to([P, nl, P])
            scrc = gh_pool.tile([P, nl * P], F32, tag="scr", name="scrc")
            nc.vector.tensor_mul(
                scrc[:].rearrange("p (i q) -> p i q", i=nl),
                gsl.rearrange("p (i q) -> p i q", i=nl), idb,
            )
            n2c = ck_pool.tile([P, nl], F32, tag="n2", name="n2c")
            nc.vector.reduce_sum(
                n2c[:], scrc[:].rearrange("p (i q) -> p i q", i=nl), axis=AXX
            )
            recc = ck_pool.tile([P, nl], F32, tag="rec", name="recc")
            nc.vector.reciprocal(recc[:], n2c[:])
            invc = ck_pool.tile([P, nl], F32, tag="invc", name="invc")
            nc.scalar.activation(invc[:], recc[:], AF.Sqrt)
            st["rec"], st["inv"] = recc, invc
            xdc = ck_pool.tile([P, nl * P], BF16, tag="xd", name="xdc")
            invb = invc[:].unsqueeze(2).broadcast_to([P, nl, P])
            nc.vector.tensor_mul(
                xdc[:].rearrange("p (i q) -> p i q", i=nl), idb, invb
            )
            st["xd"] = xdc

        def ips(ci):
            """invT = BDO^T @ xd (PE fp32r) -> Act copy -> tmp = G*invT,
            t1 = grouped rowsum (DVE); wst = tmp^2 with fused rowsum ->
            r2 (Act Square+accum); stats = (t1*inv, r2*rec) (DVE)."""
            ch = CHUNKS[ci]
            nl = len(ch)
            w = nl * P
            st = cstate[ci]
            q = ch[0] // QG
            gsl = gtiles[q][:, (ch[0] % QG) * P:((ch[0] % QG) + nl) * P]
            ipst = pp_psum.tile([P, QG * P], F32, tag="ips")
            nc.tensor.matmul(ipst[:, 0:w], bdot, st["xd"][:],
                             skip_group_check=True)
            invT = gh_pool.tile([P, nl * P], F32, tag="invT", name="invTc")
            nc.scalar.copy(invT[:], ipst[:, 0:w])
            tmpc = gh_pool.tile([P, nl * P], F32, tag="tmp", name="tmpc")
            nc.vector.tensor_mul(tmpc[:], gsl, invT[:])
            t1c = ck_pool.tile([P, nl], F32, tag="t1", name="t1c")
            nc.vector.reduce_sum(
                t1c[:], tmpc[:].rearrange("p (i q) -> p i q", i=nl), axis=AXX
            )
            stats = ck_pool.tile([P, 2 * nl], BF16, tag="stats", name="statc")
            st["stats"] = stats
            r2c = ck_pool.tile([P, nl], F32, tag="r2", name="r2c")
            for pos in range(nl):
                wsg = gh_pool.tile([P, P], F32, tag="wst")
                nc.scalar.activation(
                    wsg[:], tmpc[:, pos * P:(pos + 1) * P], AF.Square,
                    accum_out=r2c[:, pos:pos + 1],
                )
            nc.vector.tensor_mul(stats[:, 0:2 * nl:2], t1c[:], st["inv"][:])
            nc.vector.tensor_mul(stats[:, 1:2 * nl:2], r2c[:], st["rec"][:])

        gstiles = {}
        tstats = c_pool.tile([P, 2 * QG], BF16)

        def gs_copy(g):
            """Tail groups: G block -> SBUF right after its gram, so the
            tail tmp-mul can read ips from PSUM with no invT copy."""
            q, gl = divmod(g, QG)
            gsg = gh_pool.tile([P, P], F32, tag="gs", name="gsg", bufs=4)
            nc.scalar.copy(gsg[:], gtiles[q][:, gl * P:(gl + 1) * P])
            gstiles[g] = gsg

        def ips_tail(ci):
            """Tail chunks (groups 12-15): like ips() but tmp reads the
            pre-copied G from SBUF and ips from PSUM directly (no Act
            copy in the chain), and stats land in the shared tstats tile
            so one fin_tail() finishes all four groups."""
            ch = CHUNKS[ci]
            nl = len(ch)
            w = nl * P
            st = cstate[ci]
            ipst = pp_psum.tile([P, QG * P], F32, tag="ips")
            nc.tensor.matmul(ipst[:, 0:w], bdot, st["xd"][:],
                             skip_group_check=True)
            t1c = ck_pool.tile([P, nl], F32, tag="t1", name="t1c")
            r2c = ck_pool.tile([P, nl], F32, tag="r2", name="r2c")
            for pos, g in enumerate(ch):
                tmpg = gh_pool.tile([P, P], F32, tag="tmp", name="tmpg")
                nc.vector.tensor_mul(
                    tmpg[:], gstiles[g][:], ipst[:, pos * P:(pos + 1) * P]
                )
                nc.vector.reduce_sum(t1c[:, pos:pos + 1], tmpg[:], axis=AXX)
                wsg = gh_pool.tile([P, P], F32, tag="wst")
                nc.scalar.activation(
                    wsg[:], tmpg[:], AF.Square,
                    accum_out=r2c[:, pos:pos + 1],
                )
            off = ch[0] - 12
            nc.vector.tensor_mul(
                tstats[:, 2 * off:2 * (off + nl):2], t1c[:], st["inv"][:]
            )
            nc.vector.tensor_mul(
                tstats[:, 2 * off + 1:2 * (off + nl):2], r2c[:], st["rec"][:]
            )

        def fin_tail():
            """One combined finish for groups 12-15."""
            sps = j_psum.tile([P, 2 * QG], F32, tag="sps")
            nc.tensor.matmul(sps[:], bdt, tstats[:], skip_group_check=True)
            qv = sm_pool.tile([P, QG], F32, tag="qv")
            wv = sm_pool.tile([P, QG], F32, tag="wv")
            nc.scalar.activation(qv[:], sps[:, 0:2 * QG:2], AF.Square,
                                 scale=S1SCL)
            nc.scalar.mul(wv[:], sps[:, 1:2 * QG:2], -1.0 / 238.0)
            nc.vector.tensor_add(stage[:, 12:16], qv[:], wv[:])
            src = stage[:].rearrange("(b r) g -> b r g", r=V)[:, 0, 12:16]
            nc.sync.dma_start(y[:, 12:16], src)

        def fin(ci):
            """[s1',s2'] = BD^T @ stats (fp32r) -> y slice."""
            ch = CHUNKS[ci]
            st = cstate[ci]
            w = 2 * len(ch)
            sps = j_psum.tile([P, 2 * QG], F32, tag="sps")
            nc.tensor.matmul(sps[:, 0:w], bdt, st["stats"][:],
                             skip_group_check=True)
            # out = (s1*S1SCL)^2 - s2/238  (= -var)
            qv = sm_pool.tile([P, QG], F32, tag="qv")
            wv = sm_pool.tile([P, QG], F32, tag="wv")
            nl = len(ch)
            nc.scalar.activation(qv[:, 0:nl], sps[:, 0:w:2], AF.Square,
                                 scale=S1SCL)
            nc.scalar.mul(wv[:, 0:nl], sps[:, 1:w:2], -1.0 / 238.0)
            c0 = ch[0]
            nc.vector.tensor_add(stage[:, c0:c0 + nl], qv[:, 0:nl], wv[:, 0:nl])
            src = stage[:].rearrange("(b r) g -> b r g", r=V)[:, 0, c0:c0 + nl]
            nc.sync.dma_start(y[:, c0:c0 + nl], src)

        grams(0)
        post_a(0)
        grams(1)
        post_a(1)
        ips(0)
        grams(2)
        post_a(2)
        ips(1)
        fin(0)
        gram_group(12)
        gs_copy(12)
        gram_group(13)
        gs_copy(13)
        post_a(3)
        ips(2)
        gram_group(14)
        gs_copy(14)
        post_a(4)
        fin(1)
        gram_group(15)
        gs_copy(15)
        post_a(5)
        ips_tail(3)
        ips_tail(4)
        fin(2)
        ips_tail(5)
        fin_tail()


_NC_CACHE = None


def _build_nc():
    global _NC_CACHE
    if _NC_CACHE is not None:
        return _NC_CACHE
    nc = bacc.Bacc("TRN2", target_bir_lowering=False, debug=False, num_devices=NCORES)
    xh = nc.dram_tensor("x", [P, NG, NCH, P], F8, kind="ExternalInput").ap()
    cst = nc.dram_tensor("cst", [P, 3 * P], F32, kind="ExternalInput").ap()
    y = nc.dram_tensor("y", [IPG, NG], F32, kind="ExternalOutput").ap()
    with tile.TileContext(nc) as tc:
        build_tile_kernel(tc, [y], [xh, cst])
    nc.compile()
    _NC_CACHE = nc
    return nc


def make_consts():
    idn32 = np.eye(P, dtype=np.float32)
    bd = np.kron(np.eye(IPG, dtype=np.float32), np.ones((V, V), dtype=np.float32))
    bdo = bd - np.eye(P, dtype=np.float32)
    return np.ascontiguousarray(
        np.concatenate([idn32, bdo, bd], axis=1).astype(np.float32)
    )


def shard_inputs(vf):
    """vf [V*BS, C] -> list of per-core [P, NG, NCH, P] fp8 arrays with
    xh[c, g, j, v'] = row (g*128 + v') of core k's item-major layout,
    channel j*128+c. The fp8 cast is the kernel's working precision;
    pre-transposing host-side makes the device DMA fully contiguous."""
    vf3 = np.asarray(vf, dtype=np.float32).reshape(V, BS, C)
    shards = []
    for k in range(NCORES):
        sl = vf3[:, k * BS_CORE:(k + 1) * BS_CORE, :]  # [V, 128, C]
        xk = sl.transpose(1, 0, 2).reshape(BS_CORE * V, C)  # rows: item b, view v
        xk8 = xk.astype(NP_F8)
        # [g, v', j, c] -> [c, g, j, v']
        xh = xk8.reshape(NG, P, NCH, P).transpose(3, 0, 2, 1)
        shards.append(np.ascontiguousarray(xh))
    return shards


def _run(vision_features, num_views, trace=False):
    num_views = int(np.asarray(num_views))
    assert num_views == V, f"kernel hardcoded for V=16, got {num_views}"
    vf = np.asarray(vision_features, dtype=np.float32)
    assert vf.shape == (V * BS, C), vf.shape

    nc = _build_nc()
    cst = make_consts()
    shards = shard_inputs(vf)
    in_maps = [
        {"x": shards[k], "cst": cst}
        for k in range(NCORES)
    ]
    res = run_bass_kernel_spmd(
        nc, in_maps, core_ids=list(range(NCORES)), trace=trace
    )
    outs = []
    for k in range(NCORES):
        yk = res.results[k]["y"]          # [IPG, NG], y[b, g]
        outs.append(yk.T.reshape(BS_CORE))  # index g*8+b -> local item
    full = np.concatenate(outs).astype(np.float32)  # [1024]
    return full, res


def kernel(**inputs):
    out, _ = _run(**inputs)
    return out


# revision 32
# speedup vs baseline: 1.1647x; 1.1647x over previous
"""Trainium2 Bass kernel for nn_InterViews (retrieval_knn).

Computes, per batch item b: the variance (ddof=1) of the strict-upper-
triangular entries of the cosine-similarity Gram matrix between the
item's V=16 views, negated.

Strategy (data-parallel over bs across 8 cores, 128 items/core):
  - Host: shard rows so core k holds 16 groups of 8 items (each group =
    128 rows = 8 items x 16 views), cast to fp8 e4m3, and pre-transpose
    to xh[c, g, j, v] = x[g*128+v, j*128+c] so the device DMA is pure
    streaming.
  - Device DMA: one merged-const DMA, then the 16 groups as singles
    (group 0 split in 4 slices, group 1 in 2, to cut the first-gram
    latency), all on the sync HWDGE queue; every group tile stays
    SBUF-resident so nothing is gated on buffer reuse and the 16 DMA
    engines stream the 8 MB input back-to-back (~22.5 us).  Dummy fp8
    matmuls bridge the PE's DVFS ramp (0.65 -> 1.2 -> 2.4 GHz over 3 us
    of continuous execution) while the first group is in flight.
  - Grams: per group 16 fp8 DoubleRow matmuls (256 channels each)
    accumulate G = A A^T in fp32 PSUM; 4 quads ride a 4-deep PSUM ring.
  - Postproc per CHUNK (4,4,4,2,1,1 groups — quad chunks mid-stream
    where big DVE ops amortize their fixed cost, tiny chunks at the end
    so the post-stream tail chain is one group deep):
      n2   = rowsum(G * I)  (DVE mul + grouped reduce), rec = 1/n2
      inv  = sqrt(rec) (Act), xd = I*inv (DVE, fp32r)
      ips  = BDO^T @ xd     (PE, fp32r; invT pattern), Act copy to SBUF
      tmp  = G * invT (DVE), t1 = grouped rowsum (DVE)
      wst  = tmp^2 with fused rowsum -> r2 (Act Square + accum)
      stats = (t1*inv, r2*rec) (DVE, fp32r)
      [s1',s2'] = BD^T @ stats  (PE, fp32r)
      out = ((s1')*S1SCL)^2 - s2'/238 via two activations -> y DMA.
"""

import numpy as np

try:
    import concourse.bass as bass  # noqa: F401
except ImportError:  # container installs the repo at /opt/trn_rl_repo
    import sys

    sys.path.insert(0, "/opt/trn_rl_repo")

import ml_dtypes

import concourse.bass as bass
import concourse.mybir as mybir
import concourse.tile as tile
from concourse import bacc
from concourse.bass_utils import run_bass_kernel_spmd

F32 = mybir.dt.float32
F32R = mybir.dt.float32r
BF16 = mybir.dt.bfloat16
F8 = mybir.dt.float8e4
NP_F8 = ml_dtypes.float8_e4m3
P = 128          # partitions / rows per group
C = 4096         # channels
V = 16           # views per item
NCORES = 8
BS = 1024        # total batch
BS_CORE = BS // NCORES   # 128 items per core
IPG = P // V             # 8 items per group
NG = BS_CORE // IPG      # 16 groups per core
NCH = C // P             # 32 channel chunks
QG = 4                   # groups per gram PSUM quad
NQ = NG // QG

# Postproc chunks: steady-state quads, then 2+1+1 so the tail chain
# after the last gram is only one group deep.
CHUNKS = [(0, 1, 2, 3), (4, 5, 6, 7), (8, 9, 10, 11), (12, 13), (14,), (15,)]

MULT = mybir.AluOpType.mult
ADD = mybir.AluOpType.add
AF = mybir.ActivationFunctionType
AXX = mybir.AxisListType.X
DR = mybir.MatmulPerfMode.DoubleRow

S1SCL = 1.0 / np.sqrt(240.0 * 238.0)

NWARM_BIG = 3     # [128, 512] fp8 warmup matmuls (~0.5 us each)
NWARM_SMALL = 5   # [128, 128] fp8 warmup matmuls


def build_tile_kernel(tc, outs, ins):
    """ins = [xh [P, NG, NCH, P] f8e4, cst [P, 3P] f32 = [I | BD-I | BD]]
    outs = [y [IPG, NG] f32]  (y[b, g] = result for local item g*8+b)
    """
    nc = tc.nc
    xh, cst = ins
    (y,) = outs

    from contextlib import ExitStack

    with ExitStack() as ctx:
        x_pool = ctx.enter_context(tc.tile_pool(name="x", bufs=NG))
        g_psum = ctx.enter_context(tc.tile_pool(name="gp", bufs=NQ, space="PSUM"))
        pp_psum = ctx.enter_context(tc.tile_pool(name="pp", bufs=2, space="PSUM"))
        j_psum = ctx.enter_context(tc.tile_pool(name="jp", bufs=1, space="PSUM"))
        ck_pool = ctx.enter_context(tc.tile_pool(name="ckp", bufs=3))
        gh_pool = ctx.enter_context(tc.tile_pool(name="ghp", bufs=3))
        sm_pool = ctx.enter_context(tc.tile_pool(name="sm", bufs=2))
        c_pool = ctx.enter_context(tc.tile_pool(name="const", bufs=1))

        jscr = j_psum.tile([32, 32], F32)

        # Warmup scratch; output borrows the first slot of the gram PSUM
        # ring (quad 3 reuses it via same-engine ordering, no extra wait).
        wsrc = c_pool.tile([P, 4, P], F8)
        nc.vector.memset(wsrc[:], 0.0)
        wdst = g_psum.tile([P, QG * P], F32, name="gps", tag="gps")

        # Consts first (one ~0.2 us DMA), then the groups, all on sync.
        cstt = c_pool.tile([P, 3 * P], F32)
        nc.sync.dma_start(cstt[:], cst[:, :])
        ident = cstt[:, 0:P]
        # BD masks are 0/1 — exact in bf16; bf16 matmuls run at the PE
        # 1-cycle/row fast path (fp32 needs 2 half-speed passes).
        bdrr = c_pool.tile([P, 2 * P], BF16)
        nc.scalar.copy(bdrr[:], cstt[:, P:3 * P])
        bdot = bdrr[:, 0:P]
        bdt = bdrr[:, P:2 * P]

        # 16 single-group DMAs in consumption order (group 0 split in two
        # to cut first-gram latency; measured steady completion pace is
        # ~1.35 us/group and all 16 DMA engines run ~98% duty).
        xtiles = [None] * NG
        for g in range(NG):
            xg = x_pool.tile([P, NCH, P], F8, tag="x")
            if g == 0:
                nc.sync.dma_start(xg[:, 0:16, :], xh[:, g, 0:16, :])
                nc.sync.dma_start(xg[:, 16:32, :], xh[:, g, 16:32, :])
            else:
                nc.sync.dma_start(xg[:, :, :], xh[:, g, :, :])
            xtiles[g] = xg

        # PE warmup: bridges the DVFS ramp while group 0 is in flight.
        for i in range(NWARM_BIG):
            nc.tensor.matmul(wdst[:], wsrc[:, 0, :], wsrc[:],
                             skip_group_check=True)
        for i in range(NWARM_SMALL):
            nc.tensor.matmul(wdst[:, 0:P], wsrc[:, 0, :], wsrc[:, 0, :],
                             skip_group_check=True)
        # Absorb the const-DMA wait into PE's observed clock (TRN2
        # Matmult carries at most one semaphore wait).
        nc.tensor.matmul(jscr[:], cstt[0:32, 0:32], cstt[0:32, 0:32],
                         skip_group_check=True)

        stage = c_pool.tile([P, NG], F32)
        tstats = c_pool.tile([P, 2 * QG], BF16)

        gtiles = [None] * NQ
        qst = [dict() for _ in range(NQ)]   # per-quad postA state
        sst = {}                            # per-single (tail) state

        def gram_group(g):
            q, gl = divmod(g, QG)
            if gl == 0:
                gtiles[q] = g_psum.tile([P, QG * P], F32, name="gps", tag="gps")
            gps = gtiles[q]
            xg = xtiles[g]
            for j in range(NCH // 2):
                nc.tensor.matmul(
                    gps[:, gl * P:(gl + 1) * P],
                    xg[:, 2 * j:2 * j + 2, :],
                    xg[:, 2 * j:2 * j + 2, :],
                    start=(j == 0),
                    stop=(j == NCH // 2 - 1),
                    perf_mode=DR,
                    skip_group_check=True,
                )

        def post_pair(q, h):
            """Part A for pair h of quad q, issued right after its two
            grams: diag -> n2 -> rec -> inv (pair-sized ops so the chain
            after the quad's last gram is only one pair deep), and the
            pair's half of the quad xd = I*inv (bf16)."""
            st = qst[q]
            if h == 0:
                st["n2"] = ck_pool.tile([P, QG], F32, tag="n2", name="n2c")
                st["inv"] = ck_pool.tile([P, QG], F32, tag="invc", name="invc")
                st["rec"] = ck_pool.tile([P, QG], F32, tag="recc", name="recc")
                st["xd"] = ck_pool.tile([P, QG * P], BF16, tag="xd", name="xdc")
            gsl = gtiles[q][:, 2 * h * P:2 * (h + 1) * P]
            id2 = ident.unsqueeze(1).broadcast_to([P, 2, P])
            scrc = gh_pool.tile([P, 2 * P], F32, tag="scr", name="scrc")
            nc.vector.tensor_mul(
                scrc[:].rearrange("p (i q) -> p i q", i=2),
                gsl.rearrange("p (i q) -> p i q", i=2), id2,
            )
            nc.vector.reduce_sum(
                st["n2"][:, 2 * h:2 * h + 2],
                scrc[:].rearrange("p (i q) -> p i q", i=2), axis=AXX,
            )
            nc.vector.reciprocal(st["rec"][:, 2 * h:2 * h + 2],
                                 st["n2"][:, 2 * h:2 * h + 2])
            nc.scalar.activation(st["inv"][:, 2 * h:2 * h + 2],
                                 st["rec"][:, 2 * h:2 * h + 2], AF.Sqrt)
            invb = st["inv"][:, 2 * h:2 * h + 2].unsqueeze(2)
            nc.vector.tensor_mul(
                st["xd"][:, 2 * h * P:2 * (h + 1) * P]
                .rearrange("p (i q) -> p i q", i=2),
                id2, invb.broadcast_to([P, 2, P]),
            )

        def post_b(q):
            """invT = BDO^T @ xd (PE bf16), Act copy scaled by inv_row
            from PSUM -> ghw = full Ghat weights; tmp = G * ghw (DVE);
            t1 = grouped rowsum = s1 rows (DVE) directly into stats;
            r2 = rowsum(tmp^2) = s2 rows via Act Square+accum."""
            st = qst[q]
            gps = gtiles[q]
            ipst = pp_psum.tile([P, QG * P], F32, tag="ips")
            nc.tensor.matmul(ipst[:], bdot, st["xd"][:], skip_group_check=True)
            ghw = gh_pool.tile([P, QG * P], F32, tag="ghw", name="ghw")
            # scale is a per-partition AP: ghw[m, c] = inv_m * inv_c * mask
            for gl in range(QG):
                nc.scalar.activation(
                    ghw[:, gl * P:(gl + 1) * P], ipst[:, gl * P:(gl + 1) * P],
                    AF.Copy, scale=st["inv"][:, gl:gl + 1],
                )
            tmpc = gh_pool.tile([P, QG * P], F32, tag="tmp", name="tmpc")
            nc.vector.tensor_mul(tmpc[:], gps[:], ghw[:])
            stats = ck_pool.tile([P, 2 * QG], BF16, tag="stats", name="statc")
            st["stats"] = stats
            with nc.allow_low_precision(reason="bf16 stats; fp32 accum"):
                nc.vector.reduce_sum(
                    stats[:, 0:2 * QG:2],
                    tmpc[:].rearrange("p (i q) -> p i q", i=QG), axis=AXX,
                )
                for gl in range(QG):
                    wsg = gh_pool.tile([P, P], F32, tag="wst")
                    nc.scalar.activation(
                        wsg[:], tmpc[:, gl * P:(gl + 1) * P], AF.Square,
                        accum_out=stats[:, 2 * gl + 1:2 * gl + 2],
                    )

        def fin(q):
            """[s1',s2'] = BD^T @ stats (bf16) -> y slice for the quad."""
            st = qst[q]
            sps = j_psum.tile([P, 2 * QG], F32, tag="sps")
            nc.tensor.matmul(sps[:], bdt, st["stats"][:], skip_group_check=True)
            qv = sm_pool.tile([P, QG], F32, tag="qv")
            wv = sm_pool.tile([P, QG], F32, tag="wv")
            nc.scalar.activation(qv[:], sps[:, 0:2 * QG:2], AF.Square,
                                 scale=S1SCL)
            nc.vector.tensor_scalar_mul(wv[:], sps[:, 1:2 * QG:2], -1.0 / 238.0)
            nc.vector.tensor_add(stage[:, q * QG:(q + 1) * QG], qv[:], wv[:])
            src = stage[:].rearrange("(b r) g -> b r g", r=V)[:, 0,
                                                             q * QG:(q + 1) * QG]
            nc.sync.dma_start(y[:, q * QG:(q + 1) * QG], src)

        def post_a_s(g):
            """Tail single-group part A (chunk = one group)."""
            q, gl = divmod(g, QG)
            gsl = gtiles[q][:, gl * P:(gl + 1) * P]
            st = sst.setdefault(g, {})
            scrg = gh_pool.tile([P, P], F32, tag="scr", name="scrg")
            nc.vector.tensor_mul(scrg[:], gsl, ident)
            n2g = ck_pool.tile([P, 1], F32, tag="n2", name="n2g")
            nc.vector.reduce_sum(n2g[:], scrg[:], axis=AXX)
            recg = ck_pool.tile([P, 1], F32, tag="recc", name="recg")
            nc.vector.reciprocal(recg[:], n2g[:])
            invg = ck_pool.tile([P, 1], F32, tag="invc", name="invg")
            nc.scalar.activation(invg[:], recg[:], AF.Sqrt)
            st["inv"] = invg
            xdg = ck_pool.tile([P, P], BF16, tag="xd", name="xdg")
            nc.vector.tensor_mul(xdg[:], ident,
                                 invg[:].broadcast_to([P, P]))
            st["xd"] = xdg

        def post_b_s(g):
            """Tail single-group part B; stats land in shared tstats."""
            q, gl = divmod(g, QG)
            gsl = gtiles[q][:, gl * P:(gl + 1) * P]
            st = sst[g]
            ipst = pp_psum.tile([P, QG * P], F32, tag="ips")
            nc.tensor.matmul(ipst[:, 0:P], bdot, st["xd"][:],
                             skip_group_check=True)
            ghw = gh_pool.tile([P, P], F32, tag="ghw", name="ghwg")
            nc.scalar.activation(ghw[:], ipst[:, 0:P], AF.Copy,
                                 scale=st["inv"][:])
            tmpg = gh_pool.tile([P, P], F32, tag="tmp", name="tmpg")
            nc.vector.tensor_mul(tmpg[:], gsl, ghw[:])
            off = g - 12
            with nc.allow_low_precision(reason="bf16 stats; fp32 accum"):
                nc.vector.reduce_sum(tstats[:, 2 * off:2 * off + 1],
                                     tmpg[:], axis=AXX)
                wsg = gh_pool.tile([P, P], F32, tag="wst")
                nc.scalar.activation(
                    wsg[:], tmpg[:], AF.Square,
                    accum_out=tstats[:, 2 * off + 1:2 * off + 2],
                )

        def fin_tail():
            """One combined finish for groups 12-15."""
            sps = j_psum.tile([P, 2 * QG], F32, tag="sps")
            nc.tensor.matmul(sps[:], bdt, tstats[:], skip_group_check=True)
            qv = sm_pool.tile([P, QG], F32, tag="qv")
            wv = sm_pool.tile([P, QG], F32, tag="wv")
            nc.scalar.activation(qv[:], sps[:, 0:2 * QG:2], AF.Square,
                                 scale=S1SCL)
            nc.vector.tensor_scalar_mul(wv[:], sps[:, 1:2 * QG:2], -1.0 / 238.0)
            nc.vector.tensor_add(stage[:, 12:16], qv[:], wv[:])
            src = stage[:].rearrange("(b r) g -> b r g", r=V)[:, 0, 12:16]
            nc.sync.dma_start(y[:, 12:16], src)

        # Schedule: postA pairs ride right behind their grams; each
        # quad's postB (PE ips + chains) is deferred two quads so its
        # inputs are long-ready when PE reaches it; tail groups 12-15
        # run single-group chains so the post-stream tail is one group
        # deep, finished by a single merged fin.
        gram_group(0)
        gram_group(1)
        post_pair(0, 0)
        gram_group(2)
        gram_group(3)
        post_pair(0, 1)
        gram_group(4)
        gram_group(5)
        post_pair(1, 0)
        gram_group(6)
        gram_group(7)
        post_pair(1, 1)
        gram_group(8)
        post_b(0)
        gram_group(9)
        post_pair(2, 0)
        gram_group(10)
        gram_group(11)
        post_pair(2, 1)
        gram_group(12)
        post_a_s(12)
        post_b(1)
        gram_group(13)
        post_a_s(13)
        fin(0)
        gram_group(14)
        post_a_s(14)
        post_b(2)
        gram_group(15)
        post_a_s(15)
        post_b_s(12)
        post_b_s(13)
        fin(1)
        post_b_s(14)
        post_b_s(15)
        fin(2)
        fin_tail()


_NC_CACHE = None


def _build_nc():
    global _NC_CACHE
    if _NC_CACHE is not None:
        return _NC_CACHE
    nc = bacc.Bacc("TRN2", target_bir_lowering=False, debug=False, num_devices=NCORES)
    xh = nc.dram_tensor("x", [P, NG, NCH, P], F8, kind="ExternalInput").ap()
    cst = nc.dram_tensor("cst", [P, 3 * P], F32, kind="ExternalInput").ap()
    y = nc.dram_tensor("y", [IPG, NG], F32, kind="ExternalOutput").ap()
    with tile.TileContext(nc) as tc:
        build_tile_kernel(tc, [y], [xh, cst])
    nc.compile()
    _NC_CACHE = nc
    return nc


def make_consts():
    idn32 = np.eye(P, dtype=np.float32)
    bd = np.kron(np.eye(IPG, dtype=np.float32), np.ones((V, V), dtype=np.float32))
    bdo = bd - np.eye(P, dtype=np.float32)
    return np.ascontiguousarray(
        np.concatenate([idn32, bdo, bd], axis=1).astype(np.float32)
    )


def shard_inputs(vf):
    """vf [V*BS, C] -> list of per-core [P, NG, NCH, P] fp8 arrays with
    xh[c, g, j, v'] = row (g*128 + v') of core k's item-major layout,
    channel j*128+c. The fp8 cast is the kernel's working precision;
    pre-transposing host-side makes the device DMA fully contiguous."""
    vf3 = np.asarray(vf, dtype=np.float32).reshape(V, BS, C)
    shards = []
    for k in range(NCORES):
        sl = vf3[:, k * BS_CORE:(k + 1) * BS_CORE, :]  # [V, 128, C]
        xk = sl.transpose(1, 0, 2).reshape(BS_CORE * V, C)  # rows: item b, view v
        xk8 = xk.astype(NP_F8)
        # [g, v', j, c] -> [c, g, j, v']
        xh = xk8.reshape(NG, P, NCH, P).transpose(3, 0, 2, 1)
        shards.append(np.ascontiguousarray(xh))
    return shards


def _run(vision_features, num_views, trace=False):
    num_views = int(np.asarray(num_views))
    assert num_views == V, f"kernel hardcoded for V=16, got {num_views}"
    vf = np.asarray(vision_features, dtype=np.float32)
    assert vf.shape == (V * BS, C), vf.shape

    nc = _build_nc()
    cst = make_consts()
    shards = shard_inputs(vf)
    in_maps = [
        {"x": shards[k], "cst": cst}
        for k in range(NCORES)
    ]
    res = run_bass_kernel_spmd(
        nc, in_maps, core_ids=list(range(NCORES)), trace=trace
    )
    outs = []
    for k in range(NCORES):
        yk = res.results[k]["y"]          # [IPG, NG], y[b, g]
        outs.append(yk.T.reshape(BS_CORE))  # index g*8+b -> local item
    full = np.concatenate(outs).astype(np.float32)  # [1024]
    return full, res


def kernel(**inputs):
    out, _ = _run(**inputs)
    return out
